# revision 3
# baseline (speedup 1.0000x reference)
"""Chebyshev atomic descriptor kernel for 8 Trainium2 NeuronCores.

Math (matches the jax reference exactly up to fp reassociation):
  radial: per edge e with distance d:  xr = 2(d-MIN)/(RAD-MIN)-1,
    G_c = T_c(xr)*fc_rad(d), summed per atom (24 contiguous edges/atom),
    plain and typespin-weighted.
  angular: triplets are ALL pairs (j<k) of each atom's 24 edges, so
    sum_{j<k} T_m(u_j.u_k) w_j w_k = (S_m - D*T_m(1))/2 with
    S_0 = M0^2, S_1 = |M1|^2, sum c^2 ww = |M2|^2, sum c^3 ww = |M3|^2
    where Mk = sum_j w_j u_j^(x)k  (symmetric moment tensors, 1/3/6/10
    unique entries with multiplicities), D = sum_j w_j^2.
    T_2 = 2c^2-1, T_3 = 4c^3-3c  ->  linear combos of the above.
  The typespin-weighted angular sums use weights w*s (s=+-1), and the
  diagonal term D is unchanged since s^2=1.

Sharding: atoms 0..49999 padded to 50176 = 8 cores x 128 partitions x 49
atoms; each partition row owns 49 atoms x 24 contiguous edges.
"""

import numpy as np

N_ATOMS = 50000
K = 24
RAD_ORDER = 10
ANG_ORDER = 3
RAD_CUT = 8.0
ANG_CUT = 6.5
MIN_CUT = 0.55
NCORES = 8
A_PART = 49                      # atoms per partition
NA_CORE = 128 * A_PART           # 6272 atoms per core
NPAD = NCORES * NA_CORE          # 50176
NRAD = RAD_ORDER + 1             # 11
NOUT = 2 * (NRAD + ANG_ORDER + 1)  # 30

SQRT2 = float(np.sqrt(2.0))
SQRT3 = float(np.sqrt(3.0))
SQRT6 = float(np.sqrt(6.0))

_COMPILED = {}


def build_program(loop_n: int = 1):
    """Build + compile the per-core Bass program. loop_n>1 repeats the whole
    body in a hardware loop (for timing)."""
    import concourse.bacc as bacc
    import concourse.mybir as mybir
    from concourse.tile import TileContext

    f32 = mybir.dt.float32
    i32 = mybir.dt.int32
    ALU = mybir.AluOpType
    ACTF = mybir.ActivationFunctionType
    AX = mybir.AxisListType

    nc = bacc.Bacc("TRN2", target_bir_lowering=False)

    # activation() biases must come from the const-AP registry
    pi2 = float(np.pi / 2)
    _cst = nc.alloc_sbuf_tensor("const-float32-pi2", [128, 1], f32)
    nc.gpsimd.memset(_cst.ap(), pi2)
    nc.const_aps.aps[(f32, pi2)] = _cst.ap()
    nc.all_engine_barrier()

    d_dram = nc.dram_tensor("d", [128, A_PART, K], f32, kind="ExternalInput")
    u_dram = nc.dram_tensor("u", [128, A_PART, K, 3], f32, kind="ExternalInput")
    sp_dram = nc.dram_tensor("sp", [128, A_PART, K], i32, kind="ExternalInput")
    out_dram = nc.dram_tensor("out", [128, A_PART, NOUT], f32, kind="ExternalOutput")

    with TileContext(nc) as tc:
        with (
            tc.tile_pool(name="per", bufs=1) as per,      # persistent channels
            tc.tile_pool(name="rot", bufs=3) as rot,      # rotating scratch
            tc.tile_pool(name="cheb", bufs=3) as chebp,   # chebyshev chain
            tc.tile_pool(name="shared", bufs=1) as shared,  # A/B-phase shared
        ):
            def body(_iv=None):
                E3 = [128, A_PART, K]
                # ---- loads ----
                d_t = per.tile(E3, f32, tag="d")
                u_t = per.tile([128, A_PART, K, 3], f32, tag="u")
                sp_t = per.tile(E3, i32, tag="sp")
                nc.sync.dma_start(out=d_t[:, :, :], in_=d_dram.ap())
                nc.sync.dma_start(out=u_t[:, :, :, :], in_=u_dram.ap())
                nc.sync.dma_start(out=sp_t[:, :, :], in_=sp_dram.ap())

                out_t = per.tile([128, A_PART, NOUT], f32, tag="out")

                # ---- per-edge scalars ----
                # typespin: s = 2*species - 1  (int32 -> f32)
                ts_f = per.tile(E3, f32, tag="tsf")
                nc.vector.tensor_copy(out=ts_f[:, :, :], in_=sp_t[:, :, :])
                nc.vector.tensor_scalar(
                    out=ts_f[:, :, :], in0=ts_f[:, :, :],
                    scalar1=2.0, scalar2=-1.0, op0=ALU.mult, op1=ALU.add)

                # xr2 = 2*xr = (4/(RAD-MIN)) * d + 2*(-MIN*2/(RAD-MIN) - 1)
                ax = 2.0 / (RAD_CUT - MIN_CUT)
                bx = -MIN_CUT * ax - 1.0
                xr2 = per.tile(E3, f32, tag="xr2")
                nc.vector.tensor_scalar(
                    out=xr2[:, :, :], in0=d_t[:, :, :],
                    scalar1=2.0 * ax, scalar2=2.0 * bx, op0=ALU.mult, op1=ALU.add)

                # fc_rad = 0.5*cos(pi*d/RAD_CUT)+0.5  (d < RAD_CUT always)
                # cos(x) = sin(pi/2 - x); ACT Sin valid only on [-pi, pi]
                c_r = per.tile(E3, f32, tag="c_r")
                nc.scalar.activation(
                    out=c_r[:, :, :], in_=d_t[:, :, :], func=ACTF.Sin,
                    bias=float(np.pi / 2), scale=float(-np.pi / RAD_CUT))

                # fc_ang = relu(cos(pi*d/(2*ANG_CUT)))^2  (exact 0 past cut)
                c_a = per.tile(E3, f32, tag="c_a")
                nc.scalar.activation(
                    out=c_a[:, :, :], in_=d_t[:, :, :], func=ACTF.Sin,
                    bias=float(np.pi / 2), scale=float(-np.pi / (2.0 * ANG_CUT)))
                nc.scalar.activation(
                    out=c_a[:, :, :], in_=c_a[:, :, :], func=ACTF.Relu)
                w_t = per.tile(E3, f32, tag="w")
                nc.scalar.activation(
                    out=w_t[:, :, :], in_=c_a[:, :, :], func=ACTF.Square)

                # ---- radial chain: P_c = T_c(xr) * fc_rad ----
                p_prev2 = chebp.tile(E3, f32, tag="pch")
                nc.vector.tensor_scalar(
                    out=p_prev2[:, :, :], in0=c_r[:, :, :],
                    scalar1=0.5, scalar2=0.5, op0=ALU.mult, op1=ALU.add)

                p_prev1 = chebp.tile(E3, f32, tag="pch")
                nc.vector.scalar_tensor_tensor(
                    out=p_prev1[:, :, :], in0=xr2[:, :, :], scalar=0.5,
                    in1=p_prev2[:, :, :], op0=ALU.mult, op1=ALU.mult)

                def rad_out(ptile, c):
                    nc.vector.tensor_reduce(
                        out=out_t[:, :, c], in_=ptile[:, :, :],
                        axis=AX.X, op=ALU.add)
                    gw = rot.tile(E3, f32, tag="scr")
                    nc.vector.tensor_mul(gw[:, :, :], ptile[:, :, :], ts_f[:, :, :])
                    nc.vector.tensor_reduce(
                        out=out_t[:, :, NRAD + c], in_=gw[:, :, :],
                        axis=AX.X, op=ALU.add)

                rad_out(p_prev2, 0)
                rad_out(p_prev1, 1)
                for c in range(2, NRAD):
                    m = rot.tile(E3, f32, tag="scr")
                    nc.vector.tensor_mul(m[:, :, :], xr2[:, :, :], p_prev1[:, :, :])
                    p_cur = chebp.tile(E3, f32, tag="pch")
                    nc.vector.tensor_sub(p_cur[:, :, :], m[:, :, :], p_prev2[:, :, :])
                    rad_out(p_cur, c)
                    p_prev2, p_prev1 = p_prev1, p_cur

                # ---- angular moments ----
                ws_t = per.tile(E3, f32, tag="ws")
                nc.vector.tensor_mul(ws_t[:, :, :], w_t[:, :, :], ts_f[:, :, :])

                # D = sum w^2 per atom; Dh = D/2
                wsq = rot.tile(E3, f32, tag="scr")
                nc.vector.tensor_mul(wsq[:, :, :], w_t[:, :, :], w_t[:, :, :])
                dh = per.tile([128, A_PART], f32, tag="dh")
                nc.vector.tensor_reduce(
                    out=dh[:, :], in_=wsq[:, :, :], axis=AX.X, op=ALU.add)
                nc.vector.tensor_scalar_mul(dh[:, :], dh[:, :], 0.5)

                ux = u_t[:, :, :, 0]
                uy = u_t[:, :, :, 1]
                uz = u_t[:, :, :, 2]

                def moments(wgt, mom):
                    """wgt: [128,A,K] weights; mom: [128,A,20] output tile.
                    Channels: 0=M0, 1:4=M1(x,y,z), 4:7=M2 diag(xx,yy,zz),
                    7:10=M2 off(xy,xz,yz), 10:20=M3."""
                    def red(src, ch):
                        nc.vector.tensor_reduce(
                            out=mom[:, :, ch], in_=src, axis=AX.X, op=ALU.add)

                    red(wgt[:, :, :], 0)
                    p = []
                    for i, uc in enumerate((ux, uy, uz)):
                        pt = shared.tile(E3, f32, tag=f"p{i}")
                        nc.vector.tensor_mul(pt[:, :, :], wgt[:, :, :], uc)
                        red(pt[:, :, :], 1 + i)
                        p.append(pt)
                    q = {}
                    for ch, (a, b_, uc) in enumerate((
                            (0, "xx", ux), (1, "yy", uy), (2, "zz", uz),
                            (0, "xy", uy), (0, "xz", uz), (1, "yz", uz))):
                        qt = shared.tile(E3, f32, tag=f"q{b_}")
                        nc.vector.tensor_mul(qt[:, :, :], p[a][:, :, :], uc)
                        red(qt[:, :, :], 4 + ch)
                        q[b_] = qt
                    # third moments with sqrt(multiplicity) folded in
                    for ch, (qk, uc, s) in enumerate((
                            ("xx", ux, 1.0), ("yy", uy, 1.0), ("zz", uz, 1.0),
                            ("xx", uy, SQRT3), ("xx", uz, SQRT3),
                            ("yy", ux, SQRT3), ("zz", ux, SQRT3),
                            ("yy", uz, SQRT3), ("zz", uy, SQRT3),
                            ("xy", uz, SQRT6))):
                        rt = rot.tile(E3, f32, tag="scr")
                        nc.vector.scalar_tensor_tensor(
                            out=rt[:, :, :], in0=q[qk][:, :, :], scalar=s,
                            in1=uc, op0=ALU.mult, op1=ALU.mult)
                        red(rt[:, :, :], 10 + ch)
                    # sqrt(2) on off-diagonal second moments
                    nc.vector.tensor_scalar_mul(
                        mom[:, :, 7:10], mom[:, :, 7:10], SQRT2)

                def combine(mom, base):
                    """mom [128,A,20] -> out channels base..base+3."""
                    sq = shared.tile([128, A_PART, 20], f32, tag="sq")
                    nc.vector.tensor_mul(sq[:, :, :], mom[:, :, :], mom[:, :, :])
                    s1 = shared.tile([128, A_PART], f32, tag="s1")
                    nc.vector.tensor_reduce(
                        out=s1[:, :], in_=sq[:, :, 1:4], axis=AX.X, op=ALU.add)
                    c2 = shared.tile([128, A_PART], f32, tag="c2")
                    nc.vector.tensor_reduce(
                        out=c2[:, :], in_=sq[:, :, 4:10], axis=AX.X, op=ALU.add)
                    c3 = shared.tile([128, A_PART], f32, tag="c3")
                    nc.vector.tensor_reduce(
                        out=c3[:, :], in_=sq[:, :, 10:20], axis=AX.X, op=ALU.add)
                    # out0 = (S0 - D)/2
                    nc.vector.scalar_tensor_tensor(
                        out=out_t[:, :, base], in0=sq[:, :, 0], scalar=0.5,
                        in1=dh[:, :], op0=ALU.mult, op1=ALU.subtract)
                    # out1 = (S1 - D)/2
                    nc.vector.scalar_tensor_tensor(
                        out=out_t[:, :, base + 1], in0=s1[:, :], scalar=0.5,
                        in1=dh[:, :], op0=ALU.mult, op1=ALU.subtract)
                    # out2 = C2 - (S0 + D)/2
                    t1 = shared.tile([128, A_PART], f32, tag="t1")
                    nc.vector.scalar_tensor_tensor(
                        out=t1[:, :], in0=sq[:, :, 0], scalar=0.5,
                        in1=dh[:, :], op0=ALU.mult, op1=ALU.add)
                    nc.vector.tensor_sub(out_t[:, :, base + 2], c2[:, :], t1[:, :])
                    # out3 = 2*C3 - 1.5*S1 - D/2
                    t2 = shared.tile([128, A_PART], f32, tag="t2")
                    nc.vector.scalar_tensor_tensor(
                        out=t2[:, :], in0=s1[:, :], scalar=1.5,
                        in1=dh[:, :], op0=ALU.mult, op1=ALU.add)
                    nc.vector.scalar_tensor_tensor(
                        out=out_t[:, :, base + 3], in0=c3[:, :], scalar=2.0,
                        in1=t2[:, :], op0=ALU.mult, op1=ALU.subtract)

                mom_a = per.tile([128, A_PART, 20], f32, tag="momA")
                moments(w_t, mom_a)
                combine(mom_a, 2 * NRAD)
                mom_b = per.tile([128, A_PART, 20], f32, tag="momB")
                moments(ws_t, mom_b)
                combine(mom_b, 2 * NRAD + ANG_ORDER + 1)

                nc.sync.dma_start(out=out_dram.ap(), in_=out_t[:, :, :])

            if loop_n == 1:
                body()
            else:
                with tc.For_i(0, loop_n, 1) as iv:
                    body(iv)

    nc.compile()
    return nc


def _get_compiled(loop_n: int = 1):
    if loop_n not in _COMPILED:
        _COMPILED[loop_n] = build_program(loop_n)
    return _COMPILED[loop_n]


def _make_in_maps(distances, unit_vecs, neighbor_species):
    d = np.ascontiguousarray(np.asarray(distances, dtype=np.float32))
    u = np.ascontiguousarray(np.asarray(unit_vecs, dtype=np.float32))
    sp = np.ascontiguousarray(np.asarray(neighbor_species, dtype=np.int32))
    E = N_ATOMS * K
    EP = NPAD * K
    dp = np.zeros(EP, np.float32)
    dp[:E] = d
    up = np.zeros((EP, 3), np.float32)
    up[:E] = u
    spp = np.zeros(EP, np.int32)
    spp[:E] = sp
    in_maps = []
    ec = NA_CORE * K
    for c in range(NCORES):
        s = slice(c * ec, (c + 1) * ec)
        in_maps.append({
            "d": dp[s].reshape(128, A_PART, K),
            "u": up[s].reshape(128, A_PART, K, 3),
            "sp": spp[s].reshape(128, A_PART, K),
        })
    return in_maps


def run_on_hw(in_maps, loop_n: int = 1):
    from concourse.bass_utils import run_bass_kernel_spmd
    nc = _get_compiled(loop_n)
    return run_bass_kernel_spmd(nc, in_maps, core_ids=list(range(NCORES)))


def kernel(distances, unit_vecs, center_idx=None, neighbor_species=None,
           triplet_center=None, triplet_j=None, triplet_k=None,
           n_atoms=N_ATOMS, **_unused):
    in_maps = _make_in_maps(distances, unit_vecs, neighbor_species)
    res = run_on_hw(in_maps, loop_n=1)
    out = np.concatenate(
        [r["out"].reshape(NA_CORE, NOUT) for r in res.results], axis=0)
    return np.ascontiguousarray(out[:N_ATOMS])


if __name__ == "__main__":
    rng = np.random.default_rng(0)
    E = N_ATOMS * K
    d = rng.uniform(MIN_CUT + 0.05, RAD_CUT, size=E).astype(np.float32)
    v = rng.normal(size=(E, 3))
    u = (v / np.linalg.norm(v, axis=1, keepdims=True)).astype(np.float32)
    sp = rng.integers(0, 2, size=E).astype(np.int32)
    out = kernel(d, u, neighbor_species=sp)
    print(out.shape, out.dtype, out[:2])
